# revision 41
# baseline (speedup 1.0000x reference)
"""AttentionX Trainium2 kernel: 8-way head-parallel attention.

Reference computation (B=1, N=2048, C_Q=256, H=8, C_HID=32):
    q = (q_x @ Wq) * 1/sqrt(32); k = kv_x @ Wk; v = kv_x @ Wv
    scores = q k^T + attn_bias; a = softmax(scores); o = a v
    out = (o * sigmoid(q_x @ Wg)) @ Wo

Sharding: one head per NeuronCore (tensor parallel). Each core computes its
head's attention and the partial out = (o*g) @ Wo_h, plus the softmax
denominators; the host divides by the denominators and sums the partials.

Per-core layout (transposed — keys on partitions, queries on the free dim —
so the probability matrix never needs an on-chip transpose):
    qT/kT [32, 2048] f16 head-projected activations (qT pre-scaled by
    A = 1024*log2(e) so the same PSUM feeds both exp paths below).
    v natural [2048, 32] stored as vhat [128, 16*33] f16 with a ones column
    per k-tile (the ones column makes the PV matmul also emit softmax
    denominators as o_hat row 32).

The bias add is folded into the exp: exp(s+b) = exp(s)*exp(b), with exp(b)
precomputed on the host. The full transformed bias (8.4MB f16) is staged
into SBUF up front in 8 big-line chunks spread over the sync/gpsimd/scalar
DMA queues (a single queue runs at ~124GB/s with 2KB lines and would pace
the kernel).

Two passes over query halves (1024 each) so the o_hat accumulator needs
only 2 PSUM banks, leaving 6 for a 3-deep score pipeline. Per (pass,
k-tile) unit:
    PE:  scoresT = kT_i^T qT (A-scaled, f32 PSUM)
    exp: ACT exp (scale=1/A) -> es f16, then DVE mult by the exp(b) slab
         -> P f16; or (units in SCHR_UNITS) one DVE tensor_tensor add of
         the A-scaled PSUM with the host-precomputed affine bias slab,
         written int16 and bitcast to f16 — the Schraudolph bit-trick exp
         (~1.8% rms) that offloads the ACT engine.
    PE:  o_hat[33, 1024] += [v_i | 1]^T @ P_i accumulated over k-tiles.
The QK matmuls run 2 units ahead of the PV matmuls, and the v/g
projections are woven into the early units, so the tensor engine stream
stays dense (the HAM clock gate halves the PE clock within ~3.4us of
sub-saturated activity — a dense stream is worth 2x).

Gating uses tanh (same ACT table set as exp, saving a ~2.7us table swap):
sigmoid(z) = 0.5*tanh(z/2) + 0.5, applied as one DVE tensor_scalar then a
per-pass multiply of the o_hat PSUM. The gated output is scaled by 1/16 on
chip (f16 range, folded into Wv on the host) and scaled back on the host.
"""

import numpy as np

_STATE = {}

B, N, CQ, H, CH = 1, 2048, 256, 8, 32
NKT = N // 128  # 16 k-tiles
NP = 2  # query passes
HW = N // NP  # 1024 queries per pass
NU = NP * NKT  # 32 (pass, k-tile) units
OG_SCALE = 1.0 / 16.0  # folded into Wv on the host; keeps o_hat*g in f16 range
A_SCHR = 1024.0 * 1.4426950408889634  # f16-bit-trick exp: bits = A*x + C
C_SCHR = 15360.0 - 59.0  # calibrated for truncating f32->int16 conversion


# Units whose exp runs entirely on DVE via the Schraudolph bit trick
# (~1.8% rms per element; mixed over k-tiles it lands at ~1.0% output err).
SCHR_UNITS = frozenset((3, 8, 13, 19, 24, 29))
# ACT-path units whose exp(b) multiply runs on the (otherwise idle) GpSimd
# engine instead of DVE, to three-way balance the elementwise work.
GP_MULT = frozenset()


def _build_nc():
    import concourse.bacc as bacc
    import concourse.tile as tile
    from concourse import mybir

    F32 = mybir.dt.float32
    F16 = mybir.dt.float16
    I16 = mybir.dt.int16
    AF = mybir.ActivationFunctionType
    ALU = mybir.AluOpType

    nc = bacc.Bacc("TRN2", target_bir_lowering=False, debug=False, num_devices=H)

    xq_d = nc.dram_tensor("xq", [128, 2 * N], F16, kind="ExternalInput")
    xkv_d = nc.dram_tensor("xkv", [128, 2 * N], F16, kind="ExternalInput")
    w3_d = nc.dram_tensor("w3", [128, 2 * 96], F16, kind="ExternalInput")
    wv_d = nc.dram_tensor("wv", [128, 2 * 32], F16, kind="ExternalInput")
    wo_d = nc.dram_tensor("wo", [32, 256], F16, kind="ExternalInput")
    bias_d = nc.dram_tensor("biasT", [128, NU * HW], F16, kind="ExternalInput")
    # out is [p, j, c]: query q = j*128+p lives at row p, cols [256j:256j+256]
    # (big contiguous DMA lines; the host untangles the j/p interleave)
    out_d = nc.dram_tensor("out", [128, 16 * 256], F16, kind="ExternalOutput")
    sums_d = nc.dram_tensor("sums", [1, N], F32, kind="ExternalOutput")

    with tile.TileContext(nc) as tc:
        with (
            tc.tile_pool(name="const", bufs=1) as cpool,
            tc.tile_pool(name="proj", bufs=1) as ppool,
            tc.tile_pool(name="es", bufs=3) as epool,
            tc.tile_pool(name="pmat", bufs=3) as pmpool,
            tc.tile_pool(name="outs", bufs=1) as opool,
        ):
            # ---- input DMAs: inputs on the sync HW queue, bias stream on
            # the scalar HW queue (the gpsimd software-DGE queue is ~10x
            # slower and gets nothing).
            w3 = cpool.tile([128, 2 * 96], F16)
            nc.sync.dma_start(out=w3, in_=w3_d[:, :])
            wv = cpool.tile([128, 2 * 32], F16)
            nc.scalar.dma_start(out=wv, in_=wv_d[:, :])
            wo = cpool.tile([32, 256], F16)
            nc.scalar.dma_start(out=wo, in_=wo_d[:, :])
            # xq/xkv halves split across both HW queues to land sooner
            xq = cpool.tile([128, 2 * N], F16)
            nc.sync.dma_start(out=xq[:, 0:N], in_=xq_d[:, 0:N])
            nc.scalar.dma_start(out=xq[:, N : 2 * N], in_=xq_d[:, N : 2 * N])
            xkv = cpool.tile([128, 2 * N], F16)
            nc.sync.dma_start(out=xkv[:, 0:N], in_=xkv_d[:, 0:N])
            nc.scalar.dma_start(out=xkv[:, N : 2 * N], in_=xkv_d[:, N : 2 * N])
            bias_sb = cpool.tile([128, NU * HW], F16)
            CHUNK = 4 * HW  # 8KB per partition per chunk
            for c in range(NU * HW // CHUNK):
                eng = nc.scalar if c % 2 == 0 else nc.sync
                eng.dma_start(
                    out=bias_sb[:, CHUNK * c : CHUNK * (c + 1)],
                    in_=bias_d[:, CHUNK * c : CHUNK * (c + 1)],
                )

            qT = ppool.tile([32, N], F16, tag="qT")
            kT = ppool.tile([32, N], F16, tag="kT")
            tgate = ppool.tile([32, N], F16, tag="tgate")
            tp1 = ppool.tile([32, N], F16, tag="tp1")
            vhat = ppool.tile([128, NKT * 33], F16, tag="vhat")
            og = ppool.tile([32, N], F16, tag="og")
            sums_sb = ppool.tile([1, N], F32, tag="sums")
            outsb = opool.tile([128, 16 * 256], F16)

            nc.vector.memset(vhat, 1.0)

            # ---- stages 1+2 share one PSUM layout: sc ring (3 x 2 banks)
            # + o_hat (2 banks) = 8 banks. All projections run through the
            # sc ring before the attention units start cycling it.
            # PV lags QK by LOOKAHEAD units (bounded by the pt/es SBUF pool
            # depth, NOT the 3-deep PSUM sc ring — QK only needs the sc slot
            # of unit u-3 freed, which the exp does) so the cross-engine
            # exp chain (~2.5-4us) never stalls the PE.
            LOOKAHEAD = 2
            with tc.tile_pool(name="oh_ps", bufs=1, space="PSUM") as oh_pool:
              with tc.tile_pool(name="sc_ps", bufs=3, space="PSUM") as sc_pool:
                # w3 columns: [0:32]=Wq*scale*A, [32:64]=Wg, [64:96]=Wk
                # (K-slice 0), same +96 for K-slice 1.
                def emit_proj(wcol, src, dst, hh, act_copy):
                    pp = sc_pool.tile([128, HW], F32, tag="sc", name="pp")
                    for c in range(HW // 512):
                        col = HW * hh + 512 * c
                        nc.tensor.matmul(
                            pp[0:32, 512 * c : 512 * (c + 1)],
                            w3[:, wcol : wcol + 32],
                            src[:, col : col + 512],
                            start=True,
                            stop=False,
                        )
                        nc.tensor.matmul(
                            pp[0:32, 512 * c : 512 * (c + 1)],
                            w3[:, 96 + wcol : 96 + wcol + 32],
                            src[:, N + col : N + col + 512],
                            start=False,
                            stop=True,
                        )
                    d = dst[:, HW * hh : HW * (hh + 1)]
                    if act_copy == "tanh":
                        nc.scalar.activation(
                            d, pp[0:32, :], func=AF.Tanh, scale=0.5
                        )
                    elif act_copy == "act":
                        nc.scalar.copy(d, pp[0:32, :])
                    else:
                        nc.vector.tensor_copy(d, pp[0:32, :])

                with nc.named_scope("stage1_proj"):
                    for hh in range(NP):
                        emit_proj(0, xq, qT, hh, "act")
                    for hh in range(NP):
                        emit_proj(32, xq, tgate, hh, "tanh")
                    for hh in range(NP):
                        emit_proj(64, xkv, kT, hh, "dve")
                    # sigmoid(z) = 0.5*tanh(z/2) + 0.5
                    nc.vector.tensor_scalar(
                        tp1, tgate, 0.5, 0.5, ALU.mult, ALU.add
                    )
                    # v projection: all 16 k-tiles into one ring slot
                    # (columns 32r), then a single strided copy into vhat.
                    vt = sc_pool.tile([128, HW], F32, tag="sc", name="vt")
                    for r in range(NKT):
                        nc.tensor.matmul(
                            vt[:, 32 * r : 32 * (r + 1)],
                            xkv[:, 128 * r : 128 * (r + 1)],
                            wv[:, 0:32],
                            start=True,
                            stop=False,
                        )
                        nc.tensor.matmul(
                            vt[:, 32 * r : 32 * (r + 1)],
                            xkv[:, N + 128 * r : N + 128 * (r + 1)],
                            wv[:, 32:64],
                            start=False,
                            stop=True,
                        )
                    nc.vector.tensor_copy(
                        vhat.rearrange("p (r c) -> p r c", c=33)[:, :, 0:32],
                        vt[:, 0 : 32 * NKT].rearrange("p (r c) -> p r c", c=32),
                    )

                # ---- stage 2: attention, two passes over query halves ----
                # QK runs LOOKAHEAD units ahead of the PV accumulation.
                with nc.named_scope("stage2_attn"):
                    o_hats = [None, None]
                    pts = [None] * NU
                    pend = []

                    mults = [None] * NU

                    def emit_qk_exp(idx):
                        p, i = idx // NKT, idx % NKT
                        u = p * NKT + i
                        bt = bias_sb[:, HW * u : HW * (u + 1)]
                        sc = sc_pool.tile([128, HW], F32, tag="sc", name="sc")
                        # dependency-free dummy weight load: converts the
                        # short semaphore wait of the next matmul from PE
                        # idle into PE activity so the HAM clock gate stays
                        # at 8/8 (idle windows halve the PE clock)
                        nc.tensor.ldweights(kT[:, 128 * i : 128 * (i + 1)])
                        for c in range(HW // 512):
                            s = slice(512 * c, 512 * (c + 1))
                            gs = slice(HW * p + 512 * c, HW * p + 512 * (c + 1))
                            nc.tensor.matmul(
                                sc[:, s],
                                kT[:, 128 * i : 128 * (i + 1)],
                                qT[:, gs],
                                start=True,
                                stop=True,
                            )
                        pt = pmpool.tile([128, HW], F16, tag="p", name="pt")
                        if u in SCHR_UNITS:
                            # bias-add + exp fused into one DVE op per half
                            for c in range(HW // 512):
                                s = slice(512 * c, 512 * (c + 1))
                                nc.vector.tensor_tensor(
                                    pt.bitcast(I16)[:, s], sc[:, s], bt[:, s],
                                    ALU.add,
                                )
                        else:
                            es = epool.tile([128, HW], F16, tag="es", name="es")
                            nc.scalar.activation(
                                es, sc, func=AF.Exp, scale=1.0 / A_SCHR
                            )
                            # the exp-dependent multiplies are deferred one
                            # unit so the in-order DVE/GpSimd queues never
                            # wait on ACT
                            eng = nc.gpsimd if u in GP_MULT else nc.vector

                            def mult(eng=eng, pt=pt, es=es, bt=bt):
                                for c in range(HW // 512):
                                    s = slice(512 * c, 512 * (c + 1))
                                    eng.tensor_mul(pt[:, s], es[:, s], bt[:, s])

                            mults[u] = mult
                        pts[u] = pt

                    def emit_mult(idx):
                        p, i = idx // NKT, idx % NKT
                        u = p * NKT + i
                        if mults[u] is not None:
                            mults[u]()
                            mults[u] = None

                    def emit_pv(idx):
                        p, i = idx // NKT, idx % NKT
                        u = p * NKT + i
                        if i == 0:
                            o_hats[p] = oh_pool.tile(
                                [33, HW], F32, tag="oh", name="o_hat"
                            )
                        o_hat = o_hats[p]
                        pt = pts[u]
                        nc.tensor.ldweights(kT[:, 128 * i : 128 * (i + 1)])
                        for c in range(HW // 512):
                            s = slice(512 * c, 512 * (c + 1))
                            nc.tensor.matmul(
                                o_hat[:, s],
                                vhat[:, 33 * i : 33 * i + 33],
                                pt[:, s],
                                start=(i == 0),
                                stop=(i == NKT - 1),
                            )
                        pts[u] = None
                        if i == NKT - 1:
                            # gate + denominators for this pass
                            for c in range(HW // 512):
                                s = slice(HW * p + 512 * c, HW * p + 512 * (c + 1))
                                nc.vector.tensor_tensor(
                                    og[:, s],
                                    o_hat[0:32, 512 * c : 512 * (c + 1)],
                                    tp1[:, s],
                                    ALU.mult,
                                )
                            if p == 0:
                                nc.vector.tensor_copy(
                                    sums_sb[:, 0:HW], o_hat[32:33, :]
                                )
                            else:
                                nc.scalar.copy(
                                    sums_sb[:, HW:N], o_hat[32:33, :]
                                )

                    for idx in range(NU):
                        emit_qk_exp(idx)
                        if idx >= 1:
                            emit_mult(idx - 1)
                        pend.append(idx)
                        if len(pend) > LOOKAHEAD:
                            emit_pv(pend.pop(0))
                    emit_mult(NU - 1)
                    while pend:
                        emit_pv(pend.pop(0))

              # ---- stage 3: gate, denominators, output projection ----
              # (sc ring released; o_hat still held: 2 + o3 1 of 8 banks)
              with (
                  tc.tile_pool(name="o3_ps", bufs=4, space="PSUM") as o3_pool,
                  nc.named_scope("stage3_out"),
              ):
                for j in range(16):
                    ops = o3_pool.tile([128, 256], F32, tag="o3", name="ops")
                    nc.tensor.matmul(
                        ops,
                        og[:, 128 * j : 128 * (j + 1)],
                        wo,
                        start=True,
                        stop=True,
                    )
                    # alternate copy engines (both idle in the tail)
                    if j % 2 == 0:
                        nc.vector.tensor_copy(
                            outsb[:, 256 * j : 256 * (j + 1)], ops
                        )
                    else:
                        nc.scalar.copy(outsb[:, 256 * j : 256 * (j + 1)], ops)
                    if j % 4 == 3:
                        # stream each completed quarter on alternating queues
                        eng = nc.sync if (j // 4) % 2 == 0 else nc.scalar
                        qs = slice(256 * (j - 3), 256 * (j + 1))
                        eng.dma_start(out=out_d[:, qs], in_=outsb[:, qs])
                    if j == 7:
                        nc.scalar.dma_start(out=sums_d[:, :], in_=sums_sb)

    nc.compile()
    return nc


def _get_nc():
    if "nc" not in _STATE:
        _STATE["nc"] = _build_nc()
    return _STATE["nc"]


def _pack2(m, dtype):
    """[256, X] -> [128, 2X]: K-slice 0 in cols [0:X], slice 1 in [X:2X]."""
    return np.ascontiguousarray(
        np.concatenate([m[0:128], m[128:256]], axis=1).astype(dtype)
    )


def _pack_bias(bh):
    """[N, N] head bias -> [128, NU*HW] f16 slabs keyed by (pass, k-tile).

    ACT units get exp(b); SCHR units get the Schraudolph affine A*b + C.
    """
    bT = bh.T  # [keys, queries]
    out = np.empty((128, NU * HW), dtype=np.float16)
    for u in range(NU):
        p, i = u // NKT, u % NKT
        slab = bT[128 * i : 128 * (i + 1), HW * p : HW * (p + 1)]
        if u in SCHR_UNITS:
            out[:, HW * u : HW * (u + 1)] = (
                A_SCHR * slab + C_SCHR
            ).astype(np.float16)
        else:
            out[:, HW * u : HW * (u + 1)] = np.exp(slab).astype(np.float16)
    return out


def kernel(q_x, kv_x, attn_bias, Wq, Wk, Wv, Wg, Wo):
    from concourse.bass_utils import run_bass_kernel_spmd

    BF = np.float16
    nc = _get_nc()

    q_x = np.asarray(q_x, dtype=np.float32)
    kv_x = np.asarray(kv_x, dtype=np.float32)
    attn_bias = np.asarray(attn_bias, dtype=np.float32)
    Wq = np.asarray(Wq, dtype=np.float32)
    Wk = np.asarray(Wk, dtype=np.float32)
    Wv = np.asarray(Wv, dtype=np.float32)
    Wg = np.asarray(Wg, dtype=np.float32)
    Wo = np.asarray(Wo, dtype=np.float32)

    xq = _pack2(np.ascontiguousarray(q_x[0].T), BF)
    xkv = _pack2(np.ascontiguousarray(kv_x[0].T), BF)
    scale = np.float32(1.0 / np.sqrt(CH) * A_SCHR)

    in_maps = []
    for h in range(H):
        sl = slice(CH * h, CH * (h + 1))
        w3 = _pack2(
            np.concatenate([Wq[:, sl] * scale, Wg[:, sl], Wk[:, sl]], axis=1), BF
        )
        in_maps.append(
            {
                "xq": xq,
                "xkv": xkv,
                "w3": w3,
                "wv": _pack2(Wv[:, sl] * np.float32(OG_SCALE), BF),
                "wo": np.ascontiguousarray(Wo[sl, :].astype(BF)),
                "biasT": _pack_bias(attn_bias[0, h]),
            }
        )

    res = run_bass_kernel_spmd(nc, in_maps, list(range(H)))

    out = np.zeros((N, 256), dtype=np.float32)
    for h in range(H):
        # out_d is [p, j*256+c]: query q = j*128+p
        partial = (
            res.results[h]["out"]
            .astype(np.float32)
            .reshape(128, 16, 256)
            .transpose(1, 0, 2)
            .reshape(N, 256)
        )
        sums = res.results[h]["sums"][0]
        out += partial * (1.0 / OG_SCALE) / sums[:, None]
    return out.reshape(B, N, CQ).astype(np.float32)
